# revision 3
# baseline (speedup 1.0000x reference)
"""NT-Xent contrastive loss on 8 Trainium2 NeuronCores — symmetric fp8 version.

Math: z = l2-normalize rows of concat(emb_i, emb_j) -> [8192, 512].
sim = (z @ z.T)/T, T=0.5.  denom_r = sum_j exp(sim_rj) - exp(sim_rr).
loss = (sum_r log denom_r - 4*sum_k cos_k) / 8192.

exp(sim) is symmetric, so only the upper triangle of the 16x16 grid of
512-row strip pairs (136 pairs) is computed, split 17 pairs/core:
core c owns row strips A=2c, B=2c+1 and computes blocks against 10
column strips.  Per block both reductions are needed: row sums (into
denom of the row strip) and column sums (mirror, into denom of the col
strip).  Column sums: DVE accumulates exp blocks elementwise into bf16
SBUF accumulators which the host reduces over partitions.  Row sums:
the same DVE add is a scalar_tensor_tensor with accum_out, producing a
running per-partition total of the accumulator; host telescopes
consecutive totals to recover each block's row sums.  This keeps the
scalar engine (the critical resource: it must exp every computed
element, 34816/lane) free of ACTIVATION_READ_ACCUMULATOR overhead.

Device does only the O(N^2 D) work: DoubleRow fp8 matmuls (K=512 as 2
double-chunks of the [128,4,cols] ksub layout, 2x PE rate) + exp.
Host does the O(N*D) work exactly in f64: normalization, fp8(e4m3)
quantization (x64 scale; TRN FP8_EXP4 == ml_dtypes float8_e4m3 for
|v|<240), positive pairs, per-row self-term, final log/assembly.

Input DMAs are chained (corner-copy fake deps) so the first 512KB
chunk lands ASAP instead of round-robin-sharing SDMA bandwidth with
the later chunks; the narrow T8 wave runs first inside the DMA shadow.
"""

import numpy as np
import ml_dtypes

import concourse.bacc as bacc
import concourse.bass as bass
import concourse.tile as tile
from concourse import mybir
from concourse.bass_utils import run_bass_kernel_spmd

F32 = mybir.dt.float32
BF16 = mybir.dt.bfloat16
F8 = mybir.dt.float8e4
AF = mybir.ActivationFunctionType
ALU = mybir.AluOpType
ts = bass.ts

N_CORES = 8
N = 4096
D = 512
M = 2 * N
SW = 512                 # strip width (rows)
KSUB = D // 128          # 4 k-subtiles of 128
SCALE = 64.0             # fp8 quantization scale for z
ACT_SCALE = 2.0 / (SCALE * SCALE)   # exp(sim_psum * ACT_SCALE) = exp(2*cos)
NCOL = 10 * SW

# local col-tile order in zt / SBUF:  [T0 T8 | T1 T2 T3 T9 | T4 T5 T6 T7]
TILE_OFF = {0: 0, 8: 512, 1: 1024, 2: 1536, 3: 2048, 9: 2560,
            4: 3072, 5: 3584, 6: 4096, 7: 4608}
CH1, CH2, CH3 = 1024, 2048, 2048     # DMA chunk widths

# waves: (name, row(0=A,1=B), col tiles, racc slots)
WAVES = [
    ("aw2", 0, [8], [16, 17, 18, 19]),          # rowsums via DVE reduce
    ("aw0", 0, [0, 1, 2, 3], [0, 1, 2, 3]),     # full add -> accA, telescoped
    ("bw0", 1, [1, 2, 3, 9], [4, 5, 6, 7]),     # full add -> accB
    ("aw1", 0, [4, 5, 6, 7], [8, 9, 10, 11]),   # shared acc1, telescoped 0..7
    ("bw1", 1, [4, 5, 6, 7], [12, 13, 14, 15]),
]


def strips_for_core(c):
    base = [(2 * c + i) % 16 for i in range(8)]
    if c < 4:
        x, y = 2 * c + 8, 2 * c + 9
    else:
        x, y = 2 * c - 7, 2 * c - 8
    return base + [x, y]


def build_program():
    nc = bacc.Bacc(
        "TRN2",
        target_bir_lowering=False,
        debug=False,
        num_devices=N_CORES,
    )

    zt_d = nc.dram_tensor("zt", [128, KSUB, NCOL], F8, kind="ExternalInput")
    racc_d = nc.dram_tensor("racc", [128, 20], F32, kind="ExternalOutput")
    accA_d = nc.dram_tensor("accA", [128, 2048], BF16, kind="ExternalOutput")
    accB_d = nc.dram_tensor("accB", [128, 2048], BF16, kind="ExternalOutput")
    acc1_d = nc.dram_tensor("acc1", [128, 2048], BF16, kind="ExternalOutput")
    acc2_d = nc.dram_tensor("acc2", [128, 512], BF16, kind="ExternalOutput")

    DR = mybir.MatmulPerfMode.DoubleRow

    with tile.TileContext(nc) as tc:
        import contextlib

        with contextlib.ExitStack() as ctx:
            big = ctx.enter_context(tc.tile_pool(name="big", bufs=1))
            esp = ctx.enter_context(tc.tile_pool(name="esp", bufs=3))
            pp = ctx.enter_context(
                tc.tile_pool(name="pp", bufs=2, space="PSUM")
            )

            zt1 = big.tile([128, KSUB, CH1], F8, tag="zt1")
            zt2 = big.tile([128, KSUB, CH2], F8, tag="zt2")
            zt3 = big.tile([128, KSUB, CH3], F8, tag="zt3")
            accA = big.tile([128, 2048], BF16, tag="accA")
            accB = big.tile([128, 2048], BF16, tag="accB")
            acc1 = big.tile([128, 2048], BF16, tag="acc1")
            acc2 = big.tile([128, 512], BF16, tag="acc2")
            racc = big.tile([128, 20], F32, tag="racc")

            # zero accumulators on gpsimd (keeps DVE free; acc2 first,
            # it's needed earliest)
            nc.gpsimd.memset(acc2[:], 0.0)
            nc.gpsimd.memset(accA[:], 0.0)
            nc.gpsimd.memset(accB[:], 0.0)
            nc.gpsimd.memset(acc1[:], 0.0)

            # chained input DMAs: corner-copy fake deps serialize the
            # SDMA rings so chunk 1 isn't bandwidth-shared with 2 and 3
            nc.sync.dma_start(zt1[:], zt_d[:, :, 0:CH1])
            nc.vector.tensor_copy(zt2[0:1, 0:1, 0:1], zt1[0:1, 0:1, 0:1])
            nc.sync.dma_start(zt2[:], zt_d[:, :, CH1 : CH1 + CH2])
            nc.vector.tensor_copy(zt3[0:1, 0:1, 0:1], zt2[0:1, 0:1, 0:1])
            nc.sync.dma_start(zt3[:], zt_d[:, :, CH1 + CH2 : NCOL])

            def chunk_of(t):
                off = TILE_OFF[t]
                if off < CH1:
                    return zt1, off
                if off < CH1 + CH2:
                    return zt2, off - CH1
                return zt3, off - CH1 - CH2

            def emit_wave(wname, row, tiles_):
                """4 rowgroups of 128 rows from strip row(0=A,1=B) x the
                col tiles in tiles_; psum slot i = tiles_[i].  Yields
                (g, es) after the exp for each rowgroup."""
                nt = len(tiles_)
                w = nt * 512
                lcht, lbase = chunk_of(0 if row == 0 else 1)
                for g in range(4):
                    lhs_off = lbase + g * 128
                    pt = pp.tile([128, 2048], F32, tag="pp",
                                 name=f"pt_{wname}_{g}")
                    for k in range(2):
                        lhsT = lcht[:, 2 * k : 2 * k + 2,
                                    lhs_off : lhs_off + 128]
                        for i, t in enumerate(tiles_):
                            cht, choff = chunk_of(t)
                            rhs = cht[:, 2 * k : 2 * k + 2,
                                      choff : choff + 512]
                            nc.tensor.matmul(
                                pt[:, ts(i, 512)], lhsT, rhs,
                                start=(k == 0), stop=(k == 1),
                                perf_mode=DR,
                            )
                    es = esp.tile([128, 2048], BF16, tag="esp",
                                  name=f"es_{wname}_{g}")
                    nc.scalar.activation(
                        es[0:128, 0:w], pt[0:128, 0:w], AF.Exp,
                        scale=ACT_SCALE,
                    )
                    yield g, es

            def stt_add(acc_ap, es_ap, slot):
                nc.vector.scalar_tensor_tensor(
                    acc_ap, es_ap, 1.0, acc_ap,
                    ALU.mult, ALU.add,
                    accum_out=racc[:, slot : slot + 1],
                )

            # ---- A-W2 (rows A x T8), runs inside the DMA shadow ----
            for g, es in emit_wave("aw2", 0, [8]):
                nc.vector.tensor_add(acc2[:], acc2[:], es[:, 0:512])
                nc.vector.tensor_reduce(
                    racc[:, 16 + g : 17 + g], es[:, 0:512],
                    axis=mybir.AxisListType.X, op=ALU.add,
                )
            nc.sync.dma_start(acc2_d[:], acc2[:])

            # ---- A-W0: rows A x [T0 T1 T2 T3] -> accA (full width) ----
            for g, es in emit_wave("aw0", 0, [0, 1, 2, 3]):
                stt_add(accA[:], es[:, 0:2048], g)
            nc.sync.dma_start(accA_d[:], accA[:])

            # ---- B-W0: rows B x [T1 T2 T3 T9] -> accB ----
            for g, es in emit_wave("bw0", 1, [1, 2, 3, 9]):
                stt_add(accB[:], es[:, 0:2048], 4 + g)
            nc.sync.dma_start(accB_d[:], accB[:])

            # ---- A-W1 / B-W1: rows x [T4..T7] -> shared acc1 ----
            for g, es in emit_wave("aw1", 0, [4, 5, 6, 7]):
                stt_add(acc1[:], es[:, 0:2048], 8 + g)
            for g, es in emit_wave("bw1", 1, [4, 5, 6, 7]):
                stt_add(acc1[:], es[:, 0:2048], 12 + g)
            nc.sync.dma_start(acc1_d[:], acc1[:])
            nc.sync.dma_start(racc_d[:], racc[:])

    nc.compile()
    return nc


_NC_CACHE = None


def _get_program():
    global _NC_CACHE
    if _NC_CACHE is None:
        _NC_CACHE = build_program()
    return _NC_CACHE


def quantize_z(emb_i: np.ndarray, emb_j: np.ndarray):
    """Host-side exact prep: returns (q8 [8192,512] fp8, pos_sum, selfterm)."""
    reps = np.concatenate(
        [np.asarray(emb_i, np.float64), np.asarray(emb_j, np.float64)], 0
    )
    z = reps / np.linalg.norm(reps, axis=1, keepdims=True)
    q8 = (z * SCALE).astype(np.float32).astype(ml_dtypes.float8_e4m3)
    qf = q8.astype(np.float64) / SCALE
    pos_sum = float((z[:N] * z[N:]).sum())
    selfterm = np.exp(2.0 * (qf * qf).sum(1))        # device's own diag entry
    return q8, pos_sum, selfterm


def make_in_maps(q8: np.ndarray):
    # zt[p, ksub, col] = q8[global_col_row, ksub*128 + p]
    qT = np.ascontiguousarray(q8.T).reshape(KSUB, 128, M)  # [ksub, p, row]
    in_maps = []
    order_idx = sorted(TILE_OFF, key=TILE_OFF.get)   # tile ids by offset
    for c in range(N_CORES):
        S = strips_for_core(c)
        cols = np.concatenate(
            [np.arange(S[t] * SW, (S[t] + 1) * SW) for t in order_idx]
        )
        zt = np.ascontiguousarray(
            qT[:, :, cols].transpose(1, 0, 2)
        )  # [128, KSUB, NCOL]
        in_maps.append({"zt": zt})
    return in_maps


def _telescope(racc, slots):
    """Recover per-op sums from a running accumulator total."""
    prev = np.zeros(128)
    out = []
    for s in slots:
        cur = racc[:, s]
        out.append(cur - prev)
        prev = cur
    return out


def combine_outputs(results, pos_sum, selfterm):
    denom = np.zeros(M, np.float64)
    for c in range(N_CORES):
        S = strips_for_core(c)
        A, B = S[0], S[1]
        r = results[c]
        racc = np.asarray(r["racc"], np.float64)
        rowA = np.zeros((128, 4))
        rowB = np.zeros((128, 4))
        for g, v in enumerate(_telescope(racc, [0, 1, 2, 3])):     # A-W0
            rowA[:, g] += v
        for g, v in enumerate(_telescope(racc, [4, 5, 6, 7])):     # B-W0
            rowB[:, g] += v
        w1 = _telescope(racc, [8, 9, 10, 11, 12, 13, 14, 15])      # A+B W1
        for g in range(4):
            rowA[:, g] += w1[g]
            rowB[:, g] += w1[4 + g]
        rowA += racc[:, 16:20]                                     # A-W2
        denom[A * SW : (A + 1) * SW] += rowA.T.reshape(SW)
        denom[B * SW : (B + 1) * SW] += rowB.T.reshape(SW)

        csA = np.asarray(r["accA"], np.float64).sum(0)   # [T0 T1 T2 T3]
        csB = np.asarray(r["accB"], np.float64).sum(0)   # [T1 T2 T3 T9]
        cs1 = np.asarray(r["acc1"], np.float64).sum(0)   # [T4 T5 T6 T7]
        cs2 = np.asarray(r["acc2"], np.float64).sum(0)   # [T8]
        for i, t in enumerate([1, 2, 3]):                # skip T0 (diag)
            g = S[t]
            denom[g * SW : (g + 1) * SW] += csA[(i + 1) * 512 : (i + 2) * 512]
        for i, t in enumerate([2, 3, 9]):                # skip T1 (diag)
            g = S[t]
            denom[g * SW : (g + 1) * SW] += csB[(i + 1) * 512 : (i + 2) * 512]
        for i, t in enumerate([4, 5, 6, 7]):
            g = S[t]
            denom[g * SW : (g + 1) * SW] += cs1[i * 512 : (i + 1) * 512]
        g = S[8]
        denom[g * SW : (g + 1) * SW] += cs2[0:512]
    denom -= selfterm
    loss = (np.log(denom).sum() - 4.0 * pos_sum) / float(M)
    return np.float32(loss)


def kernel(emb_i: np.ndarray, emb_j: np.ndarray) -> np.ndarray:
    nc = _get_program()
    q8, pos_sum, selfterm = quantize_z(emb_i, emb_j)
    in_maps = make_in_maps(q8)
    res = run_bass_kernel_spmd(nc, in_maps, list(range(N_CORES)))
    return combine_outputs(res.results, pos_sum, selfterm)


# revision 8
# speedup vs baseline: 1.2207x; 1.2207x over previous
"""NT-Xent contrastive loss on 8 Trainium2 NeuronCores — symmetric fp8 version.

Math: z = l2-normalize rows of concat(emb_i, emb_j) -> [8192, 512].
sim = (z @ z.T)/T, T=0.5.  denom_r = sum_j exp(sim_rj) - exp(sim_rr).
loss = (sum_r log denom_r - 4*sum_k cos_k) / 8192.

exp(sim) is symmetric, so only the upper triangle of the 16x16 grid of
512-row strip pairs (136 pairs) is computed, split 17 pairs/core:
core c owns row strips A=2c, B=2c+1 and computes blocks against 10
column strips.  Per block both reductions are needed: row sums (into
denom of the row strip) and column sums (mirror, into denom of the col
strip).  Column sums: DVE accumulates exp blocks elementwise into bf16
SBUF accumulators which the host reduces over partitions.  Row sums:
the same DVE add is a scalar_tensor_tensor with accum_out, producing a
running per-partition total of the accumulator; host telescopes
consecutive totals to recover each block's row sums.  This keeps the
scalar engine (the critical resource: it must exp every computed
element, 34816/lane) free of ACTIVATION_READ_ACCUMULATOR overhead.

Device does only the O(N^2 D) work: DoubleRow fp8 matmuls (K=512 as 2
double-chunks of the [128,4,cols] ksub layout, 2x PE rate) + exp.
Host does the O(N*D) work exactly in f64: normalization, fp8(e4m3)
quantization (x64 scale; TRN FP8_EXP4 == ml_dtypes float8_e4m3 for
|v|<240), positive pairs, per-row self-term, final log/assembly.

Input DMAs are chained (corner-copy fake deps) so the first 512KB
chunk lands ASAP instead of round-robin-sharing SDMA bandwidth with
the later chunks; the narrow T8 wave runs first inside the DMA shadow.
"""

import numpy as np
import ml_dtypes

import concourse.bacc as bacc
import concourse.bass as bass
import concourse.tile as tile
from concourse import mybir
from concourse.bass_utils import run_bass_kernel_spmd

F32 = mybir.dt.float32
BF16 = mybir.dt.bfloat16
F8 = mybir.dt.float8e4
AF = mybir.ActivationFunctionType
ALU = mybir.AluOpType
ts = bass.ts

N_CORES = 8
N = 4096
D = 512
M = 2 * N
SW = 512                 # strip width (rows)
KSUB = D // 128          # 4 k-subtiles of 128
SCALE = 64.0             # fp8 quantization scale for z
ACT_SCALE = 2.0 / (SCALE * SCALE)   # exp(sim_psum * ACT_SCALE) = exp(2*cos)
NCOL = 10 * SW

# local col-tile order in zt / SBUF:  [T0 T8 | T1 T2 T3 T9 | T4 T5 T6 T7]
TILE_OFF = {0: 0, 8: 512, 1: 1024, 2: 1536, 3: 2048, 9: 2560,
            4: 3072, 5: 3584, 6: 4096, 7: 4608}
CH1, CH2, CH3 = 1024, 2048, 2048     # DMA chunk widths


def strips_for_core(c):
    base = [(2 * c + i) % 16 for i in range(8)]
    if c < 4:
        x, y = 2 * c + 8, 2 * c + 9
    else:
        x, y = 2 * c - 7, 2 * c - 8
    return base + [x, y]


def build_program():
    nc = bacc.Bacc(
        "TRN2",
        target_bir_lowering=False,
        debug=False,
        num_devices=N_CORES,
    )

    zt_d = nc.dram_tensor("zt", [128, KSUB, NCOL], F8, kind="ExternalInput")
    rs_d = nc.dram_tensor("rs", [128, 8], F32, kind="ExternalOutput")
    acc0_d = nc.dram_tensor("acc0", [128, 2048], BF16, kind="ExternalOutput")
    acc1_d = nc.dram_tensor("acc1", [128, 2048], BF16, kind="ExternalOutput")
    acc2_d = nc.dram_tensor("acc2", [128, 512], BF16, kind="ExternalOutput")

    DR = mybir.MatmulPerfMode.DoubleRow

    with tile.TileContext(nc) as tc:
        import contextlib

        with contextlib.ExitStack() as ctx:
            big = ctx.enter_context(tc.tile_pool(name="big", bufs=1))
            esp = ctx.enter_context(tc.tile_pool(name="esp", bufs=3))
            pp = ctx.enter_context(
                tc.tile_pool(name="pp", bufs=2, space="PSUM")
            )

            zt1 = big.tile([128, KSUB, CH1], F8, tag="zt1")
            zt2 = big.tile([128, KSUB, CH2], F8, tag="zt2")
            zt3 = big.tile([128, KSUB, CH3], F8, tag="zt3")
            acc0 = big.tile([128, 2048], BF16, tag="acc0")
            acc1 = big.tile([128, 2048], BF16, tag="acc1")
            acc2 = big.tile([128, 512], BF16, tag="acc2")
            dacc = big.tile([128, 32], F32, tag="dacc")
            rs = big.tile([128, 8], F32, tag="rs")

            # zero accumulators on gpsimd (keeps DVE free; acc2 and dacc
            # first, they're needed earliest)
            nc.gpsimd.memset(acc2[:], 0.0)
            nc.gpsimd.memset(dacc[:], 0.0)
            nc.gpsimd.memset(acc0[:], 0.0)
            nc.gpsimd.memset(acc1[:], 0.0)

            # chained input DMAs: corner-copy fake deps serialize the
            # SDMA rings so chunk 1 isn't bandwidth-shared with 2 and 3
            nc.sync.dma_start(zt1[:], zt_d[:, :, 0:CH1])
            nc.vector.tensor_copy(zt2[0:1, 0:1, 0:1], zt1[0:1, 0:1, 0:1])
            nc.sync.dma_start(zt2[:], zt_d[:, :, CH1 : CH1 + CH2])
            nc.vector.tensor_copy(zt3[0:1, 0:1, 0:1], zt2[0:1, 0:1, 0:1])
            nc.sync.dma_start(zt3[:], zt_d[:, :, CH1 + CH2 : NCOL])

            def chunk_of(t):
                off = TILE_OFF[t]
                if off < CH1:
                    return zt1, off
                if off < CH1 + CH2:
                    return zt2, off - CH1
                return zt3, off - CH1 - CH2

            def emit_wave(wname, row, wslot, tiles_, accum):
                """4 rowgroups of 128 rows from strip row(0=A,1=B) x the
                col tiles in tiles_; psum slot i = tiles_[i].  Yields
                (g, es) after the exp for each rowgroup.  If accum, the
                exp's accum_out writes rowsums to dacc slot
                (row*4+g)*4 + wslot."""
                nt = len(tiles_)
                w = nt * 512
                lcht, lbase = chunk_of(0 if row == 0 else 1)
                for g in range(4):
                    lhs_off = lbase + g * 128
                    pt = pp.tile([128, 2048], F32, tag="pp",
                                 name=f"pt_{wname}_{g}")
                    for k in range(2):
                        lhsT = lcht[:, 2 * k : 2 * k + 2,
                                    lhs_off : lhs_off + 128]
                        for i, t in enumerate(tiles_):
                            cht, choff = chunk_of(t)
                            rhs = cht[:, 2 * k : 2 * k + 2,
                                      choff : choff + 512]
                            nc.tensor.matmul(
                                pt[:, ts(i, 512)], lhsT, rhs,
                                start=(k == 0), stop=(k == 1),
                                perf_mode=DR,
                            )
                    es = esp.tile([128, 2048], BF16, tag="esp",
                                  name=f"es_{wname}_{g}")
                    gslot = (row * 4 + g) * 4 + wslot
                    kw = {}
                    if accum:
                        kw["accum_out"] = dacc[:, gslot : gslot + 1]
                    nc.scalar.activation(
                        es[0:128, 0:w], pt[0:128, 0:w], AF.Exp,
                        scale=ACT_SCALE, **kw,
                    )
                    yield g, gslot, es

            # ---- A-W2 (rows A x T8), runs inside the DMA shadow;
            #      rowsums via DVE reduce to keep READ_ACC off ACT ----
            for g, gslot, es in emit_wave("aw2", 0, 2, [8], accum=False):
                nc.vector.tensor_add(acc2[:], acc2[:], es[:, 0:512])
                nc.vector.tensor_reduce(
                    dacc[:, gslot : gslot + 1], es[:, 0:512],
                    axis=mybir.AxisListType.X, op=ALU.add,
                )
            nc.sync.dma_start(acc2_d[:], acc2[:])

            # ---- A-W0: rows A x [T0 T1 T2 T3]; mirrors T1..T3 ----
            for g, gslot, es in emit_wave("aw0", 0, 0, [0, 1, 2, 3],
                                          accum=True):
                nc.vector.tensor_add(
                    acc0[:, 0:1536], acc0[:, 0:1536], es[:, 512:2048]
                )
            # ---- B-W0: rows B x [T1 T2 T3 T9]; mirrors T2 T3 T9 ----
            for g, gslot, es in emit_wave("bw0", 1, 0, [1, 2, 3, 9],
                                          accum=True):
                nc.vector.tensor_add(
                    acc0[:, 512:2048], acc0[:, 512:2048], es[:, 512:2048]
                )
            nc.sync.dma_start(acc0_d[:], acc0[:])

            # ---- A-W1 / B-W1: rows x [T4..T7]; all mirrors ----
            for g, gslot, es in emit_wave("aw1", 0, 1, [4, 5, 6, 7],
                                          accum=True):
                nc.vector.tensor_add(acc1[:], acc1[:], es[:, 0:2048])
            for g, gslot, es in emit_wave("bw1", 1, 1, [4, 5, 6, 7],
                                          accum=True):
                nc.vector.tensor_add(acc1[:], acc1[:], es[:, 0:2048])
            nc.sync.dma_start(acc1_d[:], acc1[:])

            # ---- rowsum finale ----
            nc.vector.tensor_reduce(
                rs[:], dacc[:].rearrange("p (g w) -> p g w", w=4),
                axis=mybir.AxisListType.X, op=ALU.add,
            )
            nc.sync.dma_start(rs_d[:], rs[:])

    nc.compile()
    return nc


_NC_CACHE = None


def _get_program():
    global _NC_CACHE
    if _NC_CACHE is None:
        _NC_CACHE = build_program()
    return _NC_CACHE


def quantize_z(emb_i: np.ndarray, emb_j: np.ndarray):
    """Host-side exact prep: returns (q8 [8192,512] fp8, pos_sum, selfterm)."""
    reps = np.concatenate(
        [np.asarray(emb_i, np.float64), np.asarray(emb_j, np.float64)], 0
    )
    z = reps / np.linalg.norm(reps, axis=1, keepdims=True)
    q8 = (z * SCALE).astype(np.float32).astype(ml_dtypes.float8_e4m3)
    qf = q8.astype(np.float64) / SCALE
    pos_sum = float((z[:N] * z[N:]).sum())
    selfterm = np.exp(2.0 * (qf * qf).sum(1))        # device's own diag entry
    return q8, pos_sum, selfterm


def make_in_maps(q8: np.ndarray):
    # zt[p, ksub, col] = q8[global_col_row, ksub*128 + p]
    qT = np.ascontiguousarray(q8.T).reshape(KSUB, 128, M)  # [ksub, p, row]
    in_maps = []
    order_idx = sorted(TILE_OFF, key=TILE_OFF.get)   # tile ids by offset
    for c in range(N_CORES):
        S = strips_for_core(c)
        cols = np.concatenate(
            [np.arange(S[t] * SW, (S[t] + 1) * SW) for t in order_idx]
        )
        zt = np.ascontiguousarray(
            qT[:, :, cols].transpose(1, 0, 2)
        )  # [128, KSUB, NCOL]
        in_maps.append({"zt": zt})
    return in_maps


def combine_outputs(results, pos_sum, selfterm):
    denom = np.zeros(M, np.float64)
    for c in range(N_CORES):
        S = strips_for_core(c)
        A, B = S[0], S[1]
        r = results[c]
        rs = np.asarray(r["rs"], np.float64)             # [128, 8]
        denom[A * SW : (A + 1) * SW] += rs[:, 0:4].T.reshape(SW)
        denom[B * SW : (B + 1) * SW] += rs[:, 4:8].T.reshape(SW)
        cs0 = np.asarray(r["acc0"], np.float64).sum(0)   # [T1 T2 T3 T9]
        cs1 = np.asarray(r["acc1"], np.float64).sum(0)   # [T4 T5 T6 T7]
        cs2 = np.asarray(r["acc2"], np.float64).sum(0)   # [T8]
        for i, t in enumerate([1, 2, 3, 9]):
            g = S[t]
            denom[g * SW : (g + 1) * SW] += cs0[i * 512 : (i + 1) * 512]
        for i, t in enumerate([4, 5, 6, 7]):
            g = S[t]
            denom[g * SW : (g + 1) * SW] += cs1[i * 512 : (i + 1) * 512]
        g = S[8]
        denom[g * SW : (g + 1) * SW] += cs2[0:512]
    denom -= selfterm
    loss = (np.log(denom).sum() - 4.0 * pos_sum) / float(M)
    return np.float32(loss)


def kernel(emb_i: np.ndarray, emb_j: np.ndarray) -> np.ndarray:
    nc = _get_program()
    q8, pos_sum, selfterm = quantize_z(emb_i, emb_j)
    in_maps = make_in_maps(q8)
    res = run_bass_kernel_spmd(nc, in_maps, list(range(N_CORES)))
    return combine_outputs(res.results, pos_sum, selfterm)


# revision 13
# speedup vs baseline: 1.2748x; 1.0443x over previous
"""NT-Xent contrastive loss on 8 Trainium2 NeuronCores — symmetric fp8 version.

Math: z = l2-normalize rows of concat(emb_i, emb_j) -> [8192, 512].
sim = (z @ z.T)/T, T=0.5.  denom_r = sum_j exp(sim_rj) - exp(sim_rr).
loss = (sum_r log denom_r - 4*sum_k cos_k) / 8192.

exp(sim) is symmetric, so only the upper triangle of the 16x16 grid of
512-row strip pairs (136 pairs) is computed, split 17 pairs/core:
core c owns row strips A=2c, B=2c+1 and computes blocks against 10
column strips.  Per block both reductions are needed: row sums (into
denom of the row strip) and column sums (mirror, into denom of the col
strip).  Column sums: DVE accumulates exp blocks elementwise into bf16
SBUF accumulators which the host reduces over partitions.  Row sums:
the same DVE add is a scalar_tensor_tensor with accum_out, producing a
running per-partition total of the accumulator; host telescopes
consecutive totals to recover each block's row sums.  This keeps the
scalar engine (the critical resource: it must exp every computed
element, 34816/lane) free of ACTIVATION_READ_ACCUMULATOR overhead.

Device does only the O(N^2 D) work: DoubleRow fp8 matmuls (K=512 as 2
double-chunks of the [128,4,cols] ksub layout, 2x PE rate) + exp.
Host does the O(N*D) work exactly in f64: normalization, fp8(e4m3)
quantization (x64 scale; TRN FP8_EXP4 == ml_dtypes float8_e4m3 for
|v|<240), positive pairs, per-row self-term, final log/assembly.

Input DMAs are chained (corner-copy fake deps) so the first 512KB
chunk lands ASAP instead of round-robin-sharing SDMA bandwidth with
the later chunks; the narrow T8 wave runs first inside the DMA shadow.
"""

import numpy as np
import ml_dtypes

import concourse.bacc as bacc
import concourse.bass as bass
import concourse.tile as tile
from concourse import mybir
from concourse.bass_utils import run_bass_kernel_spmd

F32 = mybir.dt.float32
BF16 = mybir.dt.bfloat16
F8 = mybir.dt.float8e4
AF = mybir.ActivationFunctionType
ALU = mybir.AluOpType
ts = bass.ts

N_CORES = 8
N = 4096
D = 512
M = 2 * N
SW = 512                 # strip width (rows)
KSUB = D // 128          # 4 k-subtiles of 128
SCALE = 64.0             # fp8 quantization scale for z
ACT_SCALE = 2.0 / (SCALE * SCALE)   # exp(sim_psum * ACT_SCALE) = exp(2*cos)
NCOL = 10 * SW

# local col-tile order in zt / SBUF:  [T0 T8 | T1 T2 T3 | T9 T4 T5 T6 T7]
TILE_OFF = {0: 0, 8: 512, 1: 1024, 2: 1536, 3: 2048, 9: 2560,
            4: 3072, 5: 3584, 6: 4096, 7: 4608}
CH1, CH2, CH3 = 1024, 1536, 2560     # DMA chunk widths


def strips_for_core(c):
    base = [(2 * c + i) % 16 for i in range(8)]
    if c < 4:
        x, y = 2 * c + 8, 2 * c + 9
    else:
        x, y = 2 * c - 7, 2 * c - 8
    return base + [x, y]


def build_program():
    nc = bacc.Bacc(
        "TRN2",
        target_bir_lowering=False,
        debug=False,
        num_devices=N_CORES,
    )

    zt_d = nc.dram_tensor("zt", [128, KSUB, NCOL], F8, kind="ExternalInput")
    rs_d = nc.dram_tensor("rs", [128, 8], F32, kind="ExternalOutput")
    acc0_d = nc.dram_tensor("acc0", [128, 2048], BF16, kind="ExternalOutput")
    acc1_d = nc.dram_tensor("acc1", [128, 2048], BF16, kind="ExternalOutput")
    acc2_d = nc.dram_tensor("acc2", [128, 512], BF16, kind="ExternalOutput")

    DR = mybir.MatmulPerfMode.DoubleRow

    with tile.TileContext(nc) as tc:
        import contextlib

        with contextlib.ExitStack() as ctx:
            big = ctx.enter_context(tc.tile_pool(name="big", bufs=1))
            esp = ctx.enter_context(tc.tile_pool(name="esp", bufs=3))
            pp = ctx.enter_context(
                tc.tile_pool(name="pp", bufs=2, space="PSUM")
            )

            zt1 = big.tile([128, KSUB, CH1], F8, tag="zt1")
            zt2 = big.tile([128, KSUB, CH2], F8, tag="zt2")
            zt3 = big.tile([128, KSUB, CH3], F8, tag="zt3")
            acc0 = big.tile([128, 2048], BF16, tag="acc0")
            acc1 = big.tile([128, 2048], BF16, tag="acc1")
            acc2 = big.tile([128, 512], BF16, tag="acc2")
            dacc = big.tile([128, 32], F32, tag="dacc")
            rs = big.tile([128, 8], F32, tag="rs")

            # zero accumulators on gpsimd (keeps DVE free; acc2 and dacc
            # first, they're needed earliest)
            nc.gpsimd.memset(acc2[:], 0.0)
            nc.gpsimd.memset(dacc[:], 0.0)
            nc.gpsimd.memset(acc0[:], 0.0)
            nc.gpsimd.memset(acc1[:], 0.0)

            # chunks 1+2 stream in parallel (2-way SDMA share); chunk 3
            # has slack, so a corner-copy fake dep holds it off until
            # chunk 2 has landed instead of stealing bandwidth
            nc.sync.dma_start(zt1[:], zt_d[:, :, 0:CH1])
            nc.sync.dma_start(zt2[:], zt_d[:, :, CH1 : CH1 + CH2])
            nc.vector.tensor_copy(zt3[0:1, 0:1, 0:1], zt2[0:1, 0:1, 0:1])
            nc.sync.dma_start(zt3[:], zt_d[:, :, CH1 + CH2 : NCOL])

            def chunk_of(t):
                off = TILE_OFF[t]
                if off < CH1:
                    return zt1, off
                if off < CH1 + CH2:
                    return zt2, off - CH1
                return zt3, off - CH1 - CH2

            def emit_wave(wname, row, wslot, tiles_, accum, groups=(0, 1, 2, 3)):
                """Rowgroups of 128 rows from strip row(0=A,1=B) x the
                col tiles in tiles_; psum slot i = tiles_[i].  Yields
                (g, es) after the exp for each rowgroup.  If accum, the
                exp's accum_out writes rowsums to dacc slot
                (row*4+g)*4 + wslot."""
                nt = len(tiles_)
                w = nt * 512
                lcht, lbase = chunk_of(0 if row == 0 else 1)
                for g in groups:
                    lhs_off = lbase + g * 128
                    pt = pp.tile([128, 2048], F32, tag="pp",
                                 name=f"pt_{wname}_{g}")
                    for k in range(2):
                        lhsT = lcht[:, 2 * k : 2 * k + 2,
                                    lhs_off : lhs_off + 128]
                        for i, t in enumerate(tiles_):
                            cht, choff = chunk_of(t)
                            rhs = cht[:, 2 * k : 2 * k + 2,
                                      choff : choff + 512]
                            nc.tensor.matmul(
                                pt[:, ts(i, 512)], lhsT, rhs,
                                start=(k == 0), stop=(k == 1),
                                perf_mode=DR,
                            )
                    es = esp.tile([128, 2048], BF16, tag="esp",
                                  name=f"es_{wname}_{g}")
                    gslot = (row * 4 + g) * 4 + wslot
                    kw = {}
                    if accum:
                        kw["accum_out"] = dacc[:, gslot : gslot + 1]
                    nc.scalar.activation(
                        es[0:128, 0:w], pt[0:128, 0:w], AF.Exp,
                        scale=ACT_SCALE, **kw,
                    )
                    yield g, gslot, es

            def aw2_groups(groups):
                """A x T8 rowgroups; rowsums via DVE reduce to keep
                READ_ACC off the scalar engine."""
                for g, gslot, es in emit_wave("aw2", 0, 2, [8],
                                              accum=False, groups=groups):
                    nc.vector.tensor_add(acc2[:], acc2[:], es[:, 0:512])
                    nc.vector.tensor_reduce(
                        dacc[:, gslot : gslot + 1], es[:, 0:512],
                        axis=mybir.AxisListType.X, op=ALU.add,
                    )

            # ---- A-W2 g0 fills the head DMA shadow ----
            aw2_groups((0,))

            # ---- A-W0: rows A x [T0 T1 T2 T3]; mirrors T1..T3 ----
            for g, gslot, es in emit_wave("aw0", 0, 0, [0, 1, 2, 3],
                                          accum=True):
                nc.vector.tensor_add(
                    acc0[:, 0:1536], acc0[:, 0:1536], es[:, 512:2048]
                )
            # ---- B-W0: rows B x [T1 T2 T3 T9]; mirrors T2 T3 T9 ----
            for g, gslot, es in emit_wave("bw0", 1, 0, [1, 2, 3, 9],
                                          accum=True):
                nc.vector.tensor_add(
                    acc0[:, 512:2048], acc0[:, 512:2048], es[:, 512:2048]
                )
            nc.sync.dma_start(acc0_d[:], acc0[:])

            # ---- A-W1 / B-W1: rows x [T4..T7]; all mirrors ----
            for g, gslot, es in emit_wave("aw1", 0, 1, [4, 5, 6, 7],
                                          accum=True):
                nc.vector.tensor_add(acc1[:], acc1[:], es[:, 0:2048])
            for g, gslot, es in emit_wave("bw1", 1, 1, [4, 5, 6, 7],
                                          accum=True):
                nc.vector.tensor_add(acc1[:], acc1[:], es[:, 0:2048])
            nc.sync.dma_start(acc1_d[:], acc1[:])

            # ---- A-W2 g1..g3 close out the tail (tiny acc2 ships last) ----
            aw2_groups((1, 2, 3))
            nc.sync.dma_start(acc2_d[:], acc2[:])

            # ---- rowsum finale ----
            nc.vector.tensor_reduce(
                rs[:], dacc[:].rearrange("p (g w) -> p g w", w=4),
                axis=mybir.AxisListType.X, op=ALU.add,
            )
            nc.sync.dma_start(rs_d[:], rs[:])

    nc.compile()
    return nc


_NC_CACHE = None


def _get_program():
    global _NC_CACHE
    if _NC_CACHE is None:
        _NC_CACHE = build_program()
    return _NC_CACHE


def quantize_z(emb_i: np.ndarray, emb_j: np.ndarray):
    """Host-side exact prep: returns (q8 [8192,512] fp8, pos_sum, selfterm)."""
    reps = np.concatenate(
        [np.asarray(emb_i, np.float64), np.asarray(emb_j, np.float64)], 0
    )
    z = reps / np.linalg.norm(reps, axis=1, keepdims=True)
    q8 = (z * SCALE).astype(np.float32).astype(ml_dtypes.float8_e4m3)
    qf = q8.astype(np.float64) / SCALE
    pos_sum = float((z[:N] * z[N:]).sum())
    selfterm = np.exp(2.0 * (qf * qf).sum(1))        # device's own diag entry
    return q8, pos_sum, selfterm


def make_in_maps(q8: np.ndarray):
    # zt[p, ksub, col] = q8[global_col_row, ksub*128 + p]
    qT = np.ascontiguousarray(q8.T).reshape(KSUB, 128, M)  # [ksub, p, row]
    in_maps = []
    order_idx = sorted(TILE_OFF, key=TILE_OFF.get)   # tile ids by offset
    for c in range(N_CORES):
        S = strips_for_core(c)
        cols = np.concatenate(
            [np.arange(S[t] * SW, (S[t] + 1) * SW) for t in order_idx]
        )
        zt = np.ascontiguousarray(
            qT[:, :, cols].transpose(1, 0, 2)
        )  # [128, KSUB, NCOL]
        in_maps.append({"zt": zt})
    return in_maps


def combine_outputs(results, pos_sum, selfterm):
    denom = np.zeros(M, np.float64)
    for c in range(N_CORES):
        S = strips_for_core(c)
        A, B = S[0], S[1]
        r = results[c]
        rs = np.asarray(r["rs"], np.float64)             # [128, 8]
        denom[A * SW : (A + 1) * SW] += rs[:, 0:4].T.reshape(SW)
        denom[B * SW : (B + 1) * SW] += rs[:, 4:8].T.reshape(SW)
        cs0 = np.asarray(r["acc0"], np.float64).sum(0)   # [T1 T2 T3 T9]
        cs1 = np.asarray(r["acc1"], np.float64).sum(0)   # [T4 T5 T6 T7]
        cs2 = np.asarray(r["acc2"], np.float64).sum(0)   # [T8]
        for i, t in enumerate([1, 2, 3, 9]):
            g = S[t]
            denom[g * SW : (g + 1) * SW] += cs0[i * 512 : (i + 1) * 512]
        for i, t in enumerate([4, 5, 6, 7]):
            g = S[t]
            denom[g * SW : (g + 1) * SW] += cs1[i * 512 : (i + 1) * 512]
        g = S[8]
        denom[g * SW : (g + 1) * SW] += cs2[0:512]
    denom -= selfterm
    loss = (np.log(denom).sum() - 4.0 * pos_sum) / float(M)
    return np.float32(loss)


def kernel(emb_i: np.ndarray, emb_j: np.ndarray) -> np.ndarray:
    nc = _get_program()
    q8, pos_sum, selfterm = quantize_z(emb_i, emb_j)
    in_maps = make_in_maps(q8)
    res = run_bass_kernel_spmd(nc, in_maps, list(range(N_CORES)))
    return combine_outputs(res.results, pos_sum, selfterm)
